# revision 6
# baseline (speedup 1.0000x reference)
"""BallPooling (stride=2) Trainium2 Bass kernel, data-parallel over 8 NeuronCores.

Layout strategy (per core, 32768 leaves = 16384 balls):
  - 16 super-tiles of 1024 balls; within a super-tile, partition p holds the 8
    consecutive balls  st*1024 + p*8 + t  (t = subtile index 0..8).  Every DMA
    is therefore long-contiguous per partition (mv: 16KB runs).
  - Per 128-ball subtile: PE-transpose mv [128 balls, 512] into [(i,y), ball]
    chunks, copy PSUM->SBUF, then 4 accumulating matmuls with block-diagonal
    grade-expanded weights produce mv_out directly in [ball, (y,o)] PSUM
    layout; two more matmuls fold in the scalar path (sc channels, rel-dist,
    bias) via a 34-row transposed sc_cat.
  - EquiLayerNorm: fused square+sum (DVE tensor_tensor_reduce), bn_stats for
    the scalar LN, batched rsqrt per 3-subtile group, per-partition-scalar
    scale ops writing SBUF staging, single big DMAs out.
  - batch_out = batch_idx[::2] handled as an int32-pair gather on DVE.
"""

import json
import numpy as np

P = 128          # partitions
T = 8            # subtiles (of 128 balls) per super-tile
ST = 16          # super-tiles per core
NCORES = 8
NB_CORE = ST * T * P          # balls per core  (16384)
NL_CORE = 2 * NB_CORE         # leaves per core (32768)
EPS = 1e-5
GRADE = np.array([0, 1, 1, 1, 1, 2, 2, 2, 2, 2, 2, 3, 3, 3, 3, 4])
GROUPS = [(0, 3), (3, 6), (6, 8)]   # subtile groups for batched stats (PSUM bufs=3)


def _split_waits_json(bir_bytes: bytes, max_waits: int = 1) -> bytes:
    """This toolchain's walrus rejects instructions carrying more than one
    semaphore wait ("Too many sync wait commands").  Split extra waits into
    standalone single-wait EventSemaphore instructions on the same engine."""
    j = json.loads(bir_bytes)
    counter = [0]

    def walk(block):
        insts = block.get("instructions")
        if insts:
            new = []
            for inst in insts:
                si = inst.get("sync_info")
                ow = (si or {}).get("on_wait") or []
                if len(ow) > max_waits:
                    for w in ow[:-max_waits]:
                        counter[0] += 1
                        new.append({
                            "debug": inst.get("debug", 0),
                            "engine": inst.get("engine"),
                            "ins": [],
                            "name": f"WSPLIT-{counter[0]}",
                            "opcode": "EventSemaphore",
                            "outs": [],
                            "sync_info": {"on_update": [], "on_wait": [w]},
                        })
                    si["on_wait"] = ow[-max_waits:]
                new.append(inst)
            block["instructions"] = new
        for sub in block.get("blocks") or []:
            walk(sub)

    for fn in j["functions"]:
        for b in fn["blocks"]:
            walk(b)
    return json.dumps(j).encode()


def _patch_bass(nc):
    orig = nc.to_json_bytes

    def patched(*a, **k):
        return _split_waits_json(orig(*a, **k))

    nc.to_json_bytes = patched
    return nc


def prep_weights(w_mv, w_s2mv, w_mv2s, w_s, b_s):
    """Host-side weight rearrangement.

    Wmm[j] : [K=128, 272] for mv chunk j (i in [8j,8j+8), all y).
      K index k = il*16 + y  (il = i-8j), matching the PE transpose of the
      contiguous 128-col slice of the natural [ball, i*16+y] layout.
      cols 0..256   : c = y*16 + o   ->  w_full[o, i, y]   (block diagonal in y)
      cols 256..272 : c = 256 + o    ->  [y==0] * w_mv2s[o, i]
    Wsc : [34, 32] for the transposed sc_cat (rows: 32 sc channels, d-row,
      ones-row).  cols 0..16 -> mv grade-0 (psum cols 0..16, y=0 block);
      cols 16..32 -> sc_out (psum cols 256..272).
    """
    O, I = 16, 32
    w_full = w_mv[:, :, GRADE]                      # (16, 32, 16) [o, i, y]
    wmm = np.zeros((4, 128, 272), np.float32)
    for j in range(4):
        for il in range(8):
            i = 8 * j + il
            for y in range(16):
                k = il * 16 + y
                wmm[j, k, y * 16:y * 16 + 16] = w_full[:, i, y]
            wmm[j, il * 16 + 0, 256:272] = w_mv2s[:, i]
    wsc = np.zeros((34, 32), np.float32)
    wsc[0:32, 0:16] = w_s2mv[:, 0:32].T
    wsc[32, 0:16] = w_s2mv[:, 32] + w_s2mv[:, 33]
    wsc[0:32, 16:32] = w_s[:, 0:32].T
    wsc[32, 16:32] = w_s[:, 32] + w_s[:, 33]
    wsc[33, 16:32] = b_s
    return wmm, wsc


def build_nc():
    import concourse.bass as bass
    import concourse.tile as tile
    from concourse import mybir
    from concourse.masks import make_identity

    f32 = mybir.dt.float32
    i32 = mybir.dt.int32
    AF = mybir.ActivationFunctionType
    OP = mybir.AluOpType
    AX = mybir.AxisListType

    nc = bass.Bass("TRN2")
    mv = nc.dram_tensor("mv", (NB_CORE, 512), f32, kind="ExternalInput")
    sc = nc.dram_tensor("sc", (NB_CORE, 32), f32, kind="ExternalInput")
    pos = nc.dram_tensor("pos", (NB_CORE, 6), f32, kind="ExternalInput")
    wmm = nc.dram_tensor("wmm", (4, 128, 272), f32, kind="ExternalInput")
    wsc = nc.dram_tensor("wsc", (34, 32), f32, kind="ExternalInput")
    bidx = nc.dram_tensor("bidx", (128, 512), i32, kind="ExternalInput")
    mvn = nc.dram_tensor("mvn", (NB_CORE, 256), f32, kind="ExternalOutput")
    scn = nc.dram_tensor("scn", (NB_CORE, 16), f32, kind="ExternalOutput")
    cen = nc.dram_tensor("cen", (NB_CORE, 3), f32, kind="ExternalOutput")
    bout = nc.dram_tensor("bout", (128, 256), i32, kind="ExternalOutput")

    mv_ap = mv.rearrange("(st p t) m -> st p t m", p=P, t=T)
    sc_ap = sc.rearrange("(st p t) m -> st p t m", p=P, t=T)
    pos_ap = pos.rearrange("(st p t) m -> st p t m", p=P, t=T)
    mvn_ap = mvn.rearrange("(st p t) m -> st p t m", p=P, t=T)
    scn_ap = scn.rearrange("(st p t) m -> st p t m", p=P, t=T)
    cen_ap = cen.rearrange("(st p t) m -> st p t m", p=P, t=T)

    with tile.TileContext(nc) as tc:
        with (
            tc.tile_pool(name="consts", bufs=1) as consts,
            tc.tile_pool(name="mvin", bufs=2) as mvin,
            tc.tile_pool(name="scin", bufs=2) as scin,
            tc.tile_pool(name="posin", bufs=2) as posin,
            tc.tile_pool(name="mvt", bufs=3) as mvtp,
            tc.tile_pool(name="sct", bufs=3) as sctp,
            tc.tile_pool(name="stats", bufs=2) as stats,
            tc.tile_pool(name="stage", bufs=2) as stage_p,
            tc.tile_pool(name="scr", bufs=4) as scr,
            tc.tile_pool(name="sqp", bufs=2) as sqp,
            tc.tile_pool(name="tpsum", bufs=2, space="PSUM") as tpsum,
            tc.tile_pool(name="opsum", bufs=3, space="PSUM") as opsum,
            tc.tile_pool(name="scpsum", bufs=1, space="PSUM") as scpsum,
        ):
            ident = consts.tile([128, 128], f32)
            make_identity(nc, ident)
            eps_t = consts.tile([128, 1], f32)
            nc.vector.memset(eps_t, EPS)
            w_sb = consts.tile([128, 4, 272], f32)
            nc.sync.dma_start(out=w_sb, in_=wmm.rearrange("j k c -> k j c"))
            wsc_sb = consts.tile([34, 32], f32)
            nc.sync.dma_start(out=wsc_sb, in_=wsc[:, :])

            # ---- batch_out = batch_idx[::2] (int32 pair gather) ----
            bt = consts.tile([128, 512], i32)
            nc.sync.dma_start(out=bt, in_=bidx[:, :])
            bo = consts.tile([128, 256], i32)
            nc.vector.tensor_copy(
                out=bo.rearrange("p (b f) -> p b f", f=2),
                in_=bt.rearrange("p (b f) -> p b f", f=4)[:, :, 0:2],
            )
            nc.sync.dma_start(out=bout[:, :], in_=bo)

            for st in range(ST):
                mv_t = mvin.tile([P, T, 512], f32)
                nc.sync.dma_start(out=mv_t, in_=mv_ap[st])
                sccat = scin.tile([P, T, 34], f32)
                nc.sync.dma_start(out=sccat[:, :, 0:32], in_=sc_ap[st])
                pos_t = posin.tile([P, T, 6], f32)
                nc.sync.dma_start(out=pos_t, in_=pos_ap[st])

                # ---- geometry: centers + rel-dist (batched over all T) ----
                cen_t = stage_p.tile([P, T, 3], f32, tag="cen")
                diff = scr.tile([P, T, 3], f32, tag="diff")
                nc.vector.tensor_sub(out=diff, in0=pos_t[:, :, 0:3], in1=pos_t[:, :, 3:6])
                nc.vector.tensor_add(out=cen_t, in0=pos_t[:, :, 0:3], in1=pos_t[:, :, 3:6])
                nc.vector.tensor_scalar_mul(out=cen_t, in0=cen_t, scalar1=0.5)
                dsum = scr.tile([P, T], f32, tag="dsum")
                nc.vector.tensor_mul(out=diff, in0=diff, in1=diff)
                nc.vector.reduce_sum(out=dsum, in_=diff, axis=AX.X)
                # d = sqrt(0.25 * sum(diff^2)); both leaves share it
                nc.scalar.activation(
                    out=sccat[:, :, 32:33].rearrange("p t o -> p (t o)"),
                    in_=dsum, func=AF.Sqrt, scale=0.25)
                nc.gpsimd.memset(sccat[:, :, 33:34], 1.0)

                # ---- per-supertile stats tiles ----
                mvss = stats.tile([P, T], f32, tag="mvss")
                bnmv = stats.tile([P, T, 2], f32, tag="bnmv")
                fac = stats.tile([P, T], f32, tag="fac")
                rstd = stats.tile([P, T], f32, tag="rstd")
                stage = stage_p.tile([P, T, 256], f32, tag="mvstage")
                scstage = stage_p.tile([P, T, 16], f32, tag="scstage")

                pair_sb = {}

                def prep_pair(tp):
                    t0 = 2 * tp
                    mvt_ps = tpsum.tile([128, 1024], f32, tag="mvt_ps")
                    sct_ps = scpsum.tile([34, 256], f32, tag="sct_ps")
                    for h in range(2):
                        tt = t0 + h
                        for j in range(4):
                            nc.tensor.transpose(
                                mvt_ps[:, h * 512 + j * 128: h * 512 + (j + 1) * 128],
                                mv_t[:, tt, j * 128:(j + 1) * 128], ident)
                        nc.tensor.transpose(
                            sct_ps[:, h * 128:(h + 1) * 128], sccat[:, tt, :], ident)
                    mvt_sb = mvtp.tile([128, 1024], f32)
                    nc.scalar.copy(out=mvt_sb, in_=mvt_ps)
                    sct_sb = sctp.tile([34, 256], f32)
                    nc.vector.tensor_copy(out=sct_sb, in_=sct_ps)
                    pair_sb[tp] = (mvt_sb, sct_sb)

                def do_subtile(t):
                    mvt_sb, sct_sb = pair_sb[t // 2]
                    h = t % 2
                    ops_t = opsum.tile([128, 272], f32, tag="ops")
                    for j in range(4):
                        nc.tensor.matmul(
                            ops_t, mvt_sb[:, h * 512 + j * 128: h * 512 + (j + 1) * 128],
                            w_sb[:, j, :], start=(j == 0), stop=False)
                    sct_h = sct_sb[:, h * 128:(h + 1) * 128]
                    nc.tensor.matmul(ops_t[:, 0:16], sct_h, wsc_sb[:, 0:16],
                                     start=False, stop=False)
                    nc.tensor.matmul(ops_t[:, 256:272], sct_h, wsc_sb[:, 16:32],
                                     start=False, stop=True)
                    # stats: sum of squares over all 256 mv comps; bn stats on sc
                    sq_t = sqp.tile([128, 256], f32, tag="sq")
                    nc.scalar.activation(
                        out=sq_t, in_=ops_t[:, 0:256], func=AF.Square,
                        accum_out=mvss[:, t:t + 1])
                    bnst = scr.tile([P, 6], f32, tag="bnst")
                    nc.vector.bn_stats(out=bnst, in_=ops_t[:, 256:272])
                    nc.vector.bn_aggr(out=bnmv[:, t, :], in_=bnst)
                    return ops_t

                def finish_subtile(t, ops_t):
                    # mv_n = mv_out * fac ; psum layout (y,o) -> stage layout (o,y)
                    opsv = ops_t[:, 0:256].rearrange("p (y o) -> p y o", y=16, o=16)
                    stv = stage[:, t, :].rearrange("p (o y) -> p y o", o=16, y=16)
                    nc.vector.tensor_scalar_mul(
                        out=stv[:, 0:8, :], in0=opsv[:, 0:8, :],
                        scalar1=fac[:, t:t + 1])
                    nc.scalar.activation(
                        out=stv[:, 8:16, :], in_=opsv[:, 8:16, :],
                        func=AF.Copy, scale=fac[:, t:t + 1])
                    nc.vector.tensor_scalar(
                        out=scstage[:, t, :], in0=ops_t[:, 256:272],
                        scalar1=bnmv[:, t, 0:1],
                        scalar2=rstd[:, t:t + 1],
                        op0=OP.subtract, op1=OP.mult)

                next_pair = 0
                for (g0, g1) in GROUPS:
                    while next_pair * 2 < 2 * g1 and next_pair < T // 2:
                        if 2 * next_pair < g1:
                            prep_pair(next_pair)
                            next_pair += 1
                        else:
                            break
                    live = []
                    for t in range(g0, g1):
                        live.append((t, do_subtile(t)))
                    # batched factor math for the group
                    nc.scalar.activation(out=fac[:, g0:g1], in_=mvss[:, g0:g1],
                                         func=AF.Sqrt, scale=1.0 / 16, bias=eps_t)
                    nc.vector.reciprocal(out=fac[:, g0:g1], in_=fac[:, g0:g1])
                    nc.scalar.activation(
                        out=rstd[:, g0:g1],
                        in_=bnmv[:, g0:g1, 1:2].rearrange("p t o -> p (t o)"),
                        func=AF.Sqrt, bias=eps_t)
                    nc.vector.reciprocal(out=rstd[:, g0:g1], in_=rstd[:, g0:g1])
                    for (t, ops_t) in live:
                        finish_subtile(t, ops_t)

                nc.sync.dma_start(out=mvn_ap[st], in_=stage)
                nc.sync.dma_start(out=scn_ap[st], in_=scstage)
                nc.sync.dma_start(out=cen_ap[st], in_=cen_t)

    _patch_bass(nc)
    return nc


_NC_CACHE = None


def _get_nc():
    global _NC_CACHE
    if _NC_CACHE is None:
        _NC_CACHE = build_nc()
    return _NC_CACHE


def make_in_maps(mv, sc, pos, w_mv, w_s2mv, w_mv2s, w_s, b_s, batch_idx):
    wmm, wsc = prep_weights(
        np.asarray(w_mv, np.float32), np.asarray(w_s2mv, np.float32),
        np.asarray(w_mv2s, np.float32), np.asarray(w_s, np.float32),
        np.asarray(b_s, np.float32))
    mv = np.ascontiguousarray(mv, np.float32).reshape(-1, 256)
    sc = np.ascontiguousarray(sc, np.float32)
    pos = np.ascontiguousarray(pos, np.float32)
    bidx = np.ascontiguousarray(batch_idx).astype(np.int64, copy=False)
    in_maps = []
    for c in range(NCORES):
        L = slice(c * NL_CORE, (c + 1) * NL_CORE)
        in_maps.append({
            "mv": np.ascontiguousarray(mv[L]).reshape(NB_CORE, 512),
            "sc": np.ascontiguousarray(sc[L]).reshape(NB_CORE, 32),
            "pos": np.ascontiguousarray(pos[L]).reshape(NB_CORE, 6),
            "wmm": wmm,
            "wsc": wsc,
            "bidx": np.ascontiguousarray(bidx[L]).view(np.int32).reshape(128, 512),
        })
    return in_maps


def assemble(results):
    mv_n = np.concatenate(
        [r["mvn"].reshape(NB_CORE, 16, 16) for r in results], axis=0)
    sc_n = np.concatenate([r["scn"] for r in results], axis=0)
    centers = np.concatenate([r["cen"] for r in results], axis=0)
    batch_out = np.concatenate(
        [np.ascontiguousarray(r["bout"]).reshape(-1).view(np.int64)
         for r in results], axis=0)
    return mv_n, sc_n, centers, batch_out


def kernel(mv, sc, pos, w_mv, w_s2mv, w_mv2s, w_s, b_s, batch_idx):
    from concourse.bass_utils import run_bass_kernel_spmd

    nc = _get_nc()
    in_maps = make_in_maps(mv, sc, pos, w_mv, w_s2mv, w_mv2s, w_s, b_s, batch_idx)
    res = run_bass_kernel_spmd(nc, in_maps, core_ids=list(range(NCORES)))
    return assemble(res.results)


# revision 8
# speedup vs baseline: 37.1250x; 37.1250x over previous
"""BallPooling (stride=2) Trainium2 Bass kernel, data-parallel over 8 NeuronCores.

Layout strategy (per core, 32768 leaves = 16384 balls):
  - 16 super-tiles of 1024 balls; within a super-tile, partition p holds the 8
    consecutive balls  st*1024 + p*8 + t  (t = subtile index 0..8).  Every DMA
    is therefore long-contiguous per partition (mv: 16KB runs).
  - Per 128-ball subtile: PE-transpose mv [128 balls, 512] into [(i,y), ball]
    chunks, copy PSUM->SBUF, then 4 accumulating matmuls with block-diagonal
    grade-expanded weights produce mv_out directly in [ball, (y,o)] PSUM
    layout; two more matmuls fold in the scalar path (sc channels, rel-dist,
    bias) via a 34-row transposed sc_cat.
  - EquiLayerNorm: fused square+sum (DVE tensor_tensor_reduce), bn_stats for
    the scalar LN, batched rsqrt per 3-subtile group, per-partition-scalar
    scale ops writing SBUF staging, single big DMAs out.
  - batch_out = batch_idx[::2] handled as an int32-pair gather on DVE.
"""

import json
import numpy as np

P = 128          # partitions
T = 8            # subtiles (of 128 balls) per super-tile
ST = 16          # super-tiles per core
NCORES = 8
NB_CORE = ST * T * P          # balls per core  (16384)
NL_CORE = 2 * NB_CORE         # leaves per core (32768)
EPS = 1e-5
GRADE = np.array([0, 1, 1, 1, 1, 2, 2, 2, 2, 2, 2, 3, 3, 3, 3, 4])
GROUPS = [(0, 3), (3, 6), (6, 8)]   # subtile groups for batched stats (PSUM bufs=3)


def _split_waits_json(bir_bytes: bytes, max_waits: int = 1) -> bytes:
    """This toolchain's walrus rejects instructions carrying more than one
    semaphore wait ("Too many sync wait commands").  Split extra waits into
    standalone single-wait EventSemaphore instructions on the same engine."""
    j = json.loads(bir_bytes)
    counter = [0]

    def walk(block):
        insts = block.get("instructions")
        if insts:
            new = []
            for inst in insts:
                si = inst.get("sync_info")
                ow = (si or {}).get("on_wait") or []
                if len(ow) > max_waits:
                    for w in ow[:-max_waits]:
                        counter[0] += 1
                        new.append({
                            "debug": inst.get("debug", 0),
                            "engine": inst.get("engine"),
                            "ins": [],
                            "name": f"WSPLIT-{counter[0]}",
                            "opcode": "EventSemaphore",
                            "outs": [],
                            "sync_info": {"on_update": [], "on_wait": [w]},
                        })
                    si["on_wait"] = ow[-max_waits:]
                new.append(inst)
            block["instructions"] = new
        for sub in block.get("blocks") or []:
            walk(sub)

    for fn in j["functions"]:
        for b in fn["blocks"]:
            walk(b)
    return json.dumps(j).encode()


def _patch_bass(nc):
    orig = nc.to_json_bytes

    def patched(*a, **k):
        return _split_waits_json(orig(*a, **k))

    nc.to_json_bytes = patched
    return nc


def prep_weights(w_mv, w_s2mv, w_mv2s, w_s, b_s):
    """Host-side weight rearrangement.

    Wmm[j] : [K=128, 272] for mv chunk j (i in [8j,8j+8), all y).
      K index k = il*16 + y  (il = i-8j), matching the PE transpose of the
      contiguous 128-col slice of the natural [ball, i*16+y] layout.
      cols 0..256   : c = y*16 + o   ->  w_full[o, i, y]   (block diagonal in y)
      cols 256..272 : c = 256 + o    ->  [y==0] * w_mv2s[o, i]
    Wsc : [34, 32] for the transposed sc_cat (rows: 32 sc channels, d-row,
      ones-row).  cols 0..16 -> mv grade-0 (psum cols 0..16, y=0 block);
      cols 16..32 -> sc_out (psum cols 256..272).
    """
    O, I = 16, 32
    w_full = w_mv[:, :, GRADE]                      # (16, 32, 16) [o, i, y]
    wmm = np.zeros((4, 128, 272), np.float32)
    for j in range(4):
        for il in range(8):
            i = 8 * j + il
            for y in range(16):
                k = il * 16 + y
                wmm[j, k, y * 16:y * 16 + 16] = w_full[:, i, y]
            wmm[j, il * 16 + 0, 256:272] = w_mv2s[:, i]
    wsc = np.zeros((34, 32), np.float32)
    wsc[0:32, 0:16] = w_s2mv[:, 0:32].T
    wsc[32, 0:16] = w_s2mv[:, 32] + w_s2mv[:, 33]
    wsc[0:32, 16:32] = w_s[:, 0:32].T
    wsc[32, 16:32] = w_s[:, 32] + w_s[:, 33]
    wsc[33, 16:32] = b_s
    return wmm, wsc


def build_nc(repeat=1):
    import concourse.bass as bass
    import concourse.tile as tile
    from concourse import mybir
    from concourse.masks import make_identity

    f32 = mybir.dt.float32
    i32 = mybir.dt.int32
    AF = mybir.ActivationFunctionType
    OP = mybir.AluOpType
    AX = mybir.AxisListType

    nc = bass.Bass("TRN2")
    mv = nc.dram_tensor("mv", (NB_CORE, 512), f32, kind="ExternalInput")
    sc = nc.dram_tensor("sc", (NB_CORE, 32), f32, kind="ExternalInput")
    pos = nc.dram_tensor("pos", (NB_CORE, 6), f32, kind="ExternalInput")
    wmm = nc.dram_tensor("wmm", (4, 128, 272), f32, kind="ExternalInput")
    wsc = nc.dram_tensor("wsc", (34, 32), f32, kind="ExternalInput")
    bidx = nc.dram_tensor("bidx", (128, 512), i32, kind="ExternalInput")
    mvn = nc.dram_tensor("mvn", (NB_CORE, 256), f32, kind="ExternalOutput")
    scn = nc.dram_tensor("scn", (NB_CORE, 16), f32, kind="ExternalOutput")
    cen = nc.dram_tensor("cen", (NB_CORE, 3), f32, kind="ExternalOutput")
    bout = nc.dram_tensor("bout", (128, 256), i32, kind="ExternalOutput")

    mv_ap = mv.rearrange("(st p t) m -> st p t m", p=P, t=T)
    sc_ap = sc.rearrange("(st p t) m -> st p t m", p=P, t=T)
    pos_ap = pos.rearrange("(st p t) m -> st p t m", p=P, t=T)
    mvn_ap = mvn.rearrange("(st p t) m -> st p t m", p=P, t=T)
    scn_ap = scn.rearrange("(st p t) m -> st p t m", p=P, t=T)
    cen_ap = cen.rearrange("(st p t) m -> st p t m", p=P, t=T)

    with tile.TileContext(nc) as tc:
        with (
            tc.tile_pool(name="consts", bufs=1) as consts,
            tc.tile_pool(name="mvin", bufs=2) as mvin,
            tc.tile_pool(name="scin", bufs=2) as scin,
            tc.tile_pool(name="posin", bufs=2) as posin,
            tc.tile_pool(name="mvt", bufs=3) as mvtp,
            tc.tile_pool(name="sct", bufs=3) as sctp,
            tc.tile_pool(name="stats", bufs=2) as stats,
            tc.tile_pool(name="stage", bufs=2) as stage_p,
            tc.tile_pool(name="scr", bufs=4) as scr,
            tc.tile_pool(name="sqp", bufs=2) as sqp,
            tc.tile_pool(name="tpsum", bufs=2, space="PSUM") as tpsum,
            tc.tile_pool(name="opsum", bufs=3, space="PSUM") as opsum,
            tc.tile_pool(name="scpsum", bufs=1, space="PSUM") as scpsum,
        ):
            ident = consts.tile([128, 128], f32)
            make_identity(nc, ident)
            eps_t = consts.tile([128, 1], f32)
            nc.vector.memset(eps_t, EPS)
            w_sb = consts.tile([128, 4, 272], f32)
            nc.sync.dma_start(out=w_sb, in_=wmm.rearrange("j k c -> k j c"))
            wsc_sb = consts.tile([34, 32], f32)
            nc.sync.dma_start(out=wsc_sb, in_=wsc[:, :])

            # ---- batch_out = batch_idx[::2] (int32 pair gather) ----
            bt = consts.tile([128, 512], i32)
            nc.sync.dma_start(out=bt, in_=bidx[:, :])
            bo = consts.tile([128, 256], i32)
            nc.vector.tensor_copy(
                out=bo.rearrange("p (b f) -> p b f", f=2),
                in_=bt.rearrange("p (b f) -> p b f", f=4)[:, :, 0:2],
            )
            nc.sync.dma_start(out=bout[:, :], in_=bo)

            for st in [s for _ in range(repeat) for s in range(ST)]:
                mv_t = mvin.tile([P, T, 512], f32)
                nc.sync.dma_start(out=mv_t, in_=mv_ap[st])
                sccat = scin.tile([P, T, 34], f32)
                nc.sync.dma_start(out=sccat[:, :, 0:32], in_=sc_ap[st])
                pos_t = posin.tile([P, T, 6], f32)
                nc.sync.dma_start(out=pos_t, in_=pos_ap[st])

                # ---- geometry: centers + rel-dist (batched over all T) ----
                cen_t = stage_p.tile([P, T, 3], f32, tag="cen")
                diff = scr.tile([P, T, 3], f32, tag="diff")
                nc.vector.tensor_sub(out=diff, in0=pos_t[:, :, 0:3], in1=pos_t[:, :, 3:6])
                nc.vector.tensor_add(out=cen_t, in0=pos_t[:, :, 0:3], in1=pos_t[:, :, 3:6])
                nc.vector.tensor_scalar_mul(out=cen_t, in0=cen_t, scalar1=0.5)
                dsum = scr.tile([P, T], f32, tag="dsum")
                nc.vector.tensor_mul(out=diff, in0=diff, in1=diff)
                nc.vector.reduce_sum(out=dsum, in_=diff, axis=AX.X)
                # d = sqrt(0.25 * sum(diff^2)); both leaves share it
                nc.scalar.activation(
                    out=sccat[:, :, 32:33].rearrange("p t o -> p (t o)"),
                    in_=dsum, func=AF.Sqrt, scale=0.25)
                nc.gpsimd.memset(sccat[:, :, 33:34], 1.0)

                # ---- per-supertile stats tiles ----
                mvss = stats.tile([P, T], f32, tag="mvss")
                bnmv = stats.tile([P, T, 2], f32, tag="bnmv")
                fac = stats.tile([P, T], f32, tag="fac")
                rstd = stats.tile([P, T], f32, tag="rstd")
                stage = stage_p.tile([P, T, 256], f32, tag="mvstage")
                scstage = stage_p.tile([P, T, 16], f32, tag="scstage")

                pair_sb = {}

                def prep_pair(tp):
                    t0 = 2 * tp
                    mvt_ps = tpsum.tile([128, 1024], f32, tag="mvt_ps")
                    sct_ps = scpsum.tile([34, 256], f32, tag="sct_ps")
                    for h in range(2):
                        tt = t0 + h
                        for j in range(4):
                            nc.tensor.transpose(
                                mvt_ps[:, h * 512 + j * 128: h * 512 + (j + 1) * 128],
                                mv_t[:, tt, j * 128:(j + 1) * 128], ident)
                        nc.tensor.transpose(
                            sct_ps[:, h * 128:(h + 1) * 128], sccat[:, tt, :], ident)
                    mvt_sb = mvtp.tile([128, 1024], f32)
                    nc.scalar.copy(out=mvt_sb, in_=mvt_ps)
                    sct_sb = sctp.tile([34, 256], f32)
                    nc.vector.tensor_copy(out=sct_sb, in_=sct_ps)
                    pair_sb[tp] = (mvt_sb, sct_sb)

                def do_subtile(t):
                    mvt_sb, sct_sb = pair_sb[t // 2]
                    h = t % 2
                    ops_t = opsum.tile([128, 272], f32, tag="ops")
                    for j in range(4):
                        nc.tensor.matmul(
                            ops_t, mvt_sb[:, h * 512 + j * 128: h * 512 + (j + 1) * 128],
                            w_sb[:, j, :], start=(j == 0), stop=False)
                    sct_h = sct_sb[:, h * 128:(h + 1) * 128]
                    nc.tensor.matmul(ops_t[:, 0:16], sct_h, wsc_sb[:, 0:16],
                                     start=False, stop=False)
                    nc.tensor.matmul(ops_t[:, 256:272], sct_h, wsc_sb[:, 16:32],
                                     start=False, stop=True)
                    # stats: sum of squares over all 256 mv comps; bn stats on sc
                    sq_t = sqp.tile([128, 256], f32, tag="sq")
                    nc.scalar.activation(
                        out=sq_t, in_=ops_t[:, 0:256], func=AF.Square,
                        accum_out=mvss[:, t:t + 1])
                    bnst = scr.tile([P, 6], f32, tag="bnst")
                    nc.vector.bn_stats(out=bnst, in_=ops_t[:, 256:272])
                    nc.vector.bn_aggr(out=bnmv[:, t, :], in_=bnst)
                    return ops_t

                def finish_subtile(t, ops_t):
                    # mv_n = mv_out * fac ; psum layout (y,o) -> stage layout (o,y)
                    opsv = ops_t[:, 0:256].rearrange("p (y o) -> p y o", y=16, o=16)
                    stv = stage[:, t, :].rearrange("p (o y) -> p y o", o=16, y=16)
                    nc.vector.tensor_scalar_mul(
                        out=stv[:, 0:8, :], in0=opsv[:, 0:8, :],
                        scalar1=fac[:, t:t + 1])
                    nc.scalar.activation(
                        out=stv[:, 8:16, :], in_=opsv[:, 8:16, :],
                        func=AF.Copy, scale=fac[:, t:t + 1])
                    nc.vector.tensor_scalar(
                        out=scstage[:, t, :], in0=ops_t[:, 256:272],
                        scalar1=bnmv[:, t, 0:1],
                        scalar2=rstd[:, t:t + 1],
                        op0=OP.subtract, op1=OP.mult)

                next_pair = 0
                for (g0, g1) in GROUPS:
                    while next_pair * 2 < 2 * g1 and next_pair < T // 2:
                        if 2 * next_pair < g1:
                            prep_pair(next_pair)
                            next_pair += 1
                        else:
                            break
                    live = []
                    for t in range(g0, g1):
                        live.append((t, do_subtile(t)))
                    # batched factor math for the group
                    nc.scalar.activation(out=fac[:, g0:g1], in_=mvss[:, g0:g1],
                                         func=AF.Sqrt, scale=1.0 / 16, bias=eps_t)
                    nc.vector.reciprocal(out=fac[:, g0:g1], in_=fac[:, g0:g1])
                    nc.scalar.activation(
                        out=rstd[:, g0:g1],
                        in_=bnmv[:, g0:g1, 1:2].rearrange("p t o -> p (t o)"),
                        func=AF.Sqrt, bias=eps_t)
                    nc.vector.reciprocal(out=rstd[:, g0:g1], in_=rstd[:, g0:g1])
                    for (t, ops_t) in live:
                        finish_subtile(t, ops_t)

                nc.sync.dma_start(out=mvn_ap[st], in_=stage)
                nc.sync.dma_start(out=scn_ap[st], in_=scstage)
                nc.sync.dma_start(out=cen_ap[st], in_=cen_t)

    _patch_bass(nc)
    return nc


_NC_CACHE = None


def _get_nc():
    global _NC_CACHE
    if _NC_CACHE is None:
        _NC_CACHE = build_nc()
    return _NC_CACHE


def make_in_maps(mv, sc, pos, w_mv, w_s2mv, w_mv2s, w_s, b_s, batch_idx):
    wmm, wsc = prep_weights(
        np.asarray(w_mv, np.float32), np.asarray(w_s2mv, np.float32),
        np.asarray(w_mv2s, np.float32), np.asarray(w_s, np.float32),
        np.asarray(b_s, np.float32))
    mv = np.ascontiguousarray(mv, np.float32).reshape(-1, 256)
    sc = np.ascontiguousarray(sc, np.float32)
    pos = np.ascontiguousarray(pos, np.float32)
    bidx = np.ascontiguousarray(batch_idx).astype(np.int64, copy=False)
    in_maps = []
    for c in range(NCORES):
        L = slice(c * NL_CORE, (c + 1) * NL_CORE)
        in_maps.append({
            "mv": np.ascontiguousarray(mv[L]).reshape(NB_CORE, 512),
            "sc": np.ascontiguousarray(sc[L]).reshape(NB_CORE, 32),
            "pos": np.ascontiguousarray(pos[L]).reshape(NB_CORE, 6),
            "wmm": wmm,
            "wsc": wsc,
            "bidx": np.ascontiguousarray(bidx[L]).view(np.int32).reshape(128, 512),
        })
    return in_maps


def assemble(results):
    mv_n = np.concatenate(
        [r["mvn"].reshape(NB_CORE, 16, 16) for r in results], axis=0)
    sc_n = np.concatenate([r["scn"] for r in results], axis=0)
    centers = np.concatenate([r["cen"] for r in results], axis=0)
    batch_out = np.concatenate(
        [np.ascontiguousarray(r["bout"]).reshape(-1).view(np.int64)
         for r in results], axis=0)
    return mv_n, sc_n, centers, batch_out


def kernel(mv, sc, pos, w_mv, w_s2mv, w_mv2s, w_s, b_s, batch_idx):
    from concourse.bass_utils import run_bass_kernel_spmd

    nc = _get_nc()
    in_maps = make_in_maps(mv, sc, pos, w_mv, w_s2mv, w_mv2s, w_s, b_s, batch_idx)
    res = run_bass_kernel_spmd(nc, in_maps, core_ids=list(range(NCORES)))
    return assemble(res.results)
